# revision 1
# baseline (speedup 1.0000x reference)
"""EquiNN kernel for Trainium2 (Bass, raw), 8-core data parallel.

Computes out = l*X + g*rowsum(X) + b for X [4096, 8192] f32.
Shards X row-wise across 8 NeuronCores (512 rows each); l/g/b are baked
into the kernel as immediates at trace time (kernel compiled per call).

Raw Bass (no TileContext): this walrus build allows only one sync-wait
per DMACopy and few on the tail Drain, which Tile's auto-sem assignment
exceeds. With explicit sems every DMA carries 0 waits and every wait is
its own 1-sem instruction; there is also no Tile tail barrier (~10us).

Measured on this part: a single HWDGE ring streams only ~236 GB/s while
SWDGE (gpsimd) streams ~490 GB/s, and concurrent load+store sustains
>600 GB/s aggregate - so DMA engine placement dominates. Default config:
loads via SWDGE, stores split across both HWDGE rings (SP + ACT), rowsum
on DVE, the affine on the ACT engine, 6 SBUF slots (1.5x buffering).
"""

import os
from dataclasses import dataclass

import numpy as np

import concourse.bass as bass
from concourse import mybir
from concourse.bass_utils import run_bass_kernel_spmd

N_CORES = 8
ROWS, COLS = 4096, 8192
SHARD = ROWS // N_CORES  # 512 rows per core
P = 128                  # SBUF partitions
N_GROUPS = SHARD // P    # 4

# Filled in by kernel() when BASS_KERNEL_TRACE=1.
LAST_PROFILE = {}


@dataclass(frozen=True)
class Cfg:
    n_slots: int = 6           # SBUF x-tiles (32KB/partition each, max 6)
    loads: str = "sw"          # 'sw' (gpsimd SWDGE) | 'sp' | 'act'  (HWDGE)
    stores: tuple = ("sp", "act")  # round-robin over these engines
    affine: str = "act"        # 'act' | 'dve'
    compute: bool = True       # False => store straight after load (DMA floor)


DEFAULT_CFG = Cfg()


def _build(
    l: float, g: float, b: float, reps: int = 1, cfg: Cfg = DEFAULT_CFG
) -> bass.Bass:
    nc = bass.Bass()
    X = nc.declare_dram_parameter("X", [SHARD, COLS], mybir.dt.float32, isOutput=False)
    out = nc.declare_dram_parameter("out", [SHARD, COLS], mybir.dt.float32, isOutput=True)

    Xg = X.rearrange("(gr p) c -> gr p c", p=P)
    outg = out.rearrange("(gr p) c -> gr p c", p=P)

    f32 = mybir.dt.float32
    ns = cfg.n_slots
    n_idx = reps * N_GROUPS

    import contextlib

    with contextlib.ExitStack() as ctx:
        xt = [
            ctx.enter_context(nc.sbuf_tensor(f"xt{i}", [P, COLS], f32))
            for i in range(ns)
        ]
        rs = [
            ctx.enter_context(nc.sbuf_tensor(f"rs{i}", [P, 1], f32))
            for i in range(ns)
        ]
        s = [
            ctx.enter_context(nc.sbuf_tensor(f"s{i}", [P, 1], f32))
            for i in range(ns)
        ]
        load_sems = [
            ctx.enter_context(nc.semaphore(f"load_sem{i}")) for i in range(ns)
        ]
        store_sems = [
            ctx.enter_context(nc.semaphore(f"store_sem{i}")) for i in range(ns)
        ]
        act_sems = [
            ctx.enter_context(nc.semaphore(f"act_sem{i}")) for i in range(ns)
        ]
        dve_sem = ctx.enter_context(nc.semaphore("dve_sem"))
        block = ctx.enter_context(nc.Block())

        # occupancy bookkeeping: idx = r*N_GROUPS + g runs through slots
        # round-robin; prior(idx) = how many earlier tiles used this slot.
        def slot(idx):
            return idx % ns

        def prior(idx):
            return idx // ns

        def total(sl):
            return (n_idx - 1 - sl) // ns + 1 if sl < n_idx else 0

        def engine_fn(kind):
            return {"sw": block.gpsimd, "sp": block.sync, "act": block.scalar}[kind]

        # ---- load engine ----------------------------------------------
        def load_prog(eng):
            for idx in range(n_idx):
                sl, pr, g_ = slot(idx), prior(idx), idx % N_GROUPS
                if pr > 0:
                    eng.wait_ge(store_sems[sl], 16 * pr)
                eng.dma_start(xt[sl][:], Xg[g_]).then_inc(load_sems[sl], 16)
            # final barrier: all stores landed before the program ends
            for sl in range(min(ns, n_idx)):
                eng.wait_ge(store_sems[sl], 16 * total(sl))

        # ---- store engines --------------------------------------------
        def store_prog(eng, eng_i, n_engs):
            for idx in range(n_idx):
                if idx % n_engs != eng_i:
                    continue
                sl, pr, g_ = slot(idx), prior(idx), idx % N_GROUPS
                if cfg.compute:
                    eng.wait_ge(act_sems[sl], pr + 1)
                else:
                    eng.wait_ge(load_sems[sl], 16 * (pr + 1))
                eng.dma_start(outg[g_], xt[sl][:]).then_inc(store_sems[sl], 16)

        # ---- DVE: rowsum + s = g*rs + b (+ affine if cfg.affine=='dve')
        def dve_prog(vector):
            for idx in range(n_idx):
                sl, pr = slot(idx), prior(idx)
                vector.wait_ge(load_sems[sl], 16 * (pr + 1))
                if idx >= 1:
                    # serialize DVE (deep pipeline; also guards rs/s WAR)
                    vector.wait_ge(dve_sem, 2 * idx)
                nc.vector.reduce_sum(
                    rs[sl][:], xt[sl][:], axis=mybir.AxisListType.X
                ).then_inc(dve_sem, 1)
                vector.wait_ge(dve_sem, 2 * idx + 1)
                if pr > 0:
                    # s[sl] may still be read by affine of the previous
                    # occupant when affine runs on ACT
                    vector.wait_ge(act_sems[sl], pr)
                nc.vector.tensor_scalar(
                    s[sl][:], rs[sl][:], g, b,
                    op0=mybir.AluOpType.mult, op1=mybir.AluOpType.add,
                ).then_inc(dve_sem, 1)
                if cfg.affine == "dve":
                    vector.wait_ge(dve_sem, 2 * idx + 2)
                    nc.vector.tensor_scalar(
                        xt[sl][:], xt[sl][:], l, s[sl][:],
                        op0=mybir.AluOpType.mult, op1=mybir.AluOpType.add,
                    ).then_inc(act_sems[sl], 1)

        # ---- ACT: affine x = l*x + s ----------------------------------
        def act_prog(scalar):
            for idx in range(n_idx):
                sl = slot(idx)
                scalar.wait_ge(dve_sem, 2 * idx + 2)
                nc.scalar.activation(
                    xt[sl][:], xt[sl][:],
                    mybir.ActivationFunctionType.Identity,
                    bias=s[sl][:], scale=l,
                ).then_inc(act_sems[sl], 1)

        # ---- wire the engine programs ---------------------------------
        # (sequential emitters would deadlock if loads shared an engine
        # with stores: all load preps would precede all store preps)
        assert cfg.loads not in cfg.stores, "loads/stores must use distinct engines"
        progs = {}  # engine kind -> list of emitters, in order

        progs.setdefault(cfg.loads, []).append(load_prog)
        if cfg.compute:
            progs.setdefault("dve", []).append(dve_prog)
            if cfg.affine == "act":
                progs.setdefault("act", []).append(act_prog)
        n_store_engs = len(cfg.stores)
        for i, se in enumerate(cfg.stores):
            progs.setdefault(se, []).append(
                lambda eng, i=i: store_prog(eng, i, n_store_engs)
            )

        # each engine gets exactly one block function running its emitters
        def make(fns):
            def _prog(eng):
                for f in fns:
                    f(eng)

            return _prog

        for kind, fns in progs.items():
            if kind == "dve":
                block.vector(make(fns))
            elif kind == "act":
                block.scalar(make(fns))
            else:
                engine_fn(kind)(make(fns))

    return nc


def kernel(X: np.ndarray, l: np.ndarray, g: np.ndarray, b: np.ndarray) -> np.ndarray:
    nc = _build(float(l[0]), float(g[0]), float(b[0]))

    shards = np.ascontiguousarray(X, dtype=np.float32).reshape(N_CORES, SHARD, COLS)
    in_maps = [{"X": shards[i]} for i in range(N_CORES)]

    trace = os.environ.get("BASS_KERNEL_TRACE") == "1"
    res = run_bass_kernel_spmd(nc, in_maps, list(range(N_CORES)), trace=trace)
    if trace:
        LAST_PROFILE.update(
            exec_time_ns=res.exec_time_ns,
            mean_exec_time_ns=res.mean_exec_time_ns,
            trace=res.instructions_and_trace[1] if res.instructions_and_trace else None,
            profile_json=res.profile_json,
        )
    return np.concatenate([res.results[i]["out"] for i in range(N_CORES)], axis=0)

